# revision 29
# baseline (speedup 1.0000x reference)
"""Trainium2 Bass kernel for KL-divergence attention.

Math used (exactly equivalent to the reference model):
  q = x@Wq, k = x@Wk, v = x@Wv
  kl_ij  = sum_h p_i log p_i - p_i . logq_j   (p = softmax(q), logq = log_softmax(k))
  attn   = softmax_j(-kl_ij) = softmax_j(p_i . logq_j)     [neg-entropy cancels]
         = softmax_j(p_i . k_j - lse_j)
  With exp(s - lse_j) = exp(s)/sk_j (sk_j = sum_h exp(k_jh)), the 1/sk_j
  factor is absorbed into the V rows and the softmax-denominator matmul, so
  no log is needed:
    e'_ij = exp(p_i . k_j)
    out_i = (sum_j e'_ij * (v_j/sk_j)) / (sum_j e'_ij / sk_j)

Layout / precision strategy (per core, 4 of the 32 batches, data-parallel):
  - fp16 operands for all projection / attention-output matmuls (same PE rate
    as bf16, 8x finer mantissa); fp32 PSUM accumulation everywhere
  - the pairwise scores GEMM runs in fp8e4 with perf_mode=DoubleRow
    (K=256 per matmul, 2x ALU): p is pre-scaled by 256 to sit in fp8's
    normal range, undone for free via the activation's scale on exp
  - scores are computed TRANSPOSED (sT[j,i] = kT.T @ pT) so exp(scores)
    feeds the output GEMM as its stationary operand with no transpose
  - all 128x128 transposes are REGULAR matmuls against an identity (or a
    diag(256/sq) for p, folding the softmax normalization into the
    transpose) — transpose-mode does not engage the PE clock-gate and runs
    at ~1.2 GHz; regular matmuls run warm at 2.4 GHz
"""

import numpy as np

import concourse.bass as bass
import concourse.tile as tile
from concourse import bacc, mybir
from concourse.bass_utils import run_bass_kernel_spmd
from concourse.masks import make_identity

B, S, D, H = 32, 2048, 512, 512
NCORES = 8
BPC = B // NCORES  # batches per core
P = 128
NB = S // P   # 16 row blocks per batch
ND = D // P   # 4 contraction chunks
NH = H // P   # 4 h chunks
NG = 4        # i groups in phase 3
GW = S // NG  # 512 i columns per group

FP32 = mybir.dt.float32
FP16 = mybir.dt.float16
FP8 = mybir.dt.float8e4
EXP = mybir.ActivationFunctionType.Exp
DR = mybir.MatmulPerfMode.DoubleRow

# Scores-GEMM precision. fp8+DoubleRow is ~2x faster on the pairwise GEMM
# (~100us end-to-end) but raises output absmax error from ~0.8e-3 to ~8e-3;
# fp16 keeps full accuracy margin.
USE_FP8_SCORES = True
SCORES_DT = FP8 if USE_FP8_SCORES else FP16
PS = 256.0 if USE_FP8_SCORES else 1.0  # p pre-scale for fp8 normal range

# q/k projections run in fp8+DoubleRow. Wq/Wk are host-scaled by WS (their
# entries are ~N(0, 1/512), far below fp8e4's normal range); the exp undoes
# it via the activation scale. v stays fp16 (v-quantization noise does not
# average out in the output, q/k noise only perturbs attention logits).
USE_FP8_PROJ = True
WS = 32.0
QS = WS if USE_FP8_PROJ else 1.0  # scale sitting on the q/k psum logits

# fp8+DoubleRow output GEMM via an exact mean-deviation split:
#   out_i = (sum_j c_j vr_j + sum_j (e'_ij - c_j) vr_j) / (same split on rsk)
# with e' = exp(scores), vr = v/sk, and c_j ~= mean_i e'_ij taken from the
# scores-exp's free accum_out. The deviation g = e' - c is ~10x smaller than
# e', so its fp8 quantization noise lands on the *deviation* scale; the bulk
# paths (w = sum_j c_j vr16_j and l0 = sum_j c_j rsk16_j) stay fp16, which is
# what kills the fp8 quantization bias (v/rsk noise does not average out).
# The denominator rides along as column 256 of the first of two half-width
# output psums (rsk appended to the fp8 v tiles), so it costs no extra
# matmuls and no separate N=1 chain.
USE_FP8_OUT = True
VS = 1024.0  # fp8 v-tile pre-scale: v/sk ~ 1e-3 sits below fp8e4's range
HA = 257     # output psum A width: 256 v columns + the denominator column
HB = H - 256


def _emit(tc):
    # Inputs arrive pre-sharded/pre-laid-out by the host side of kernel():
    # x as [BPC, D, S] fp16 (transposed so the contraction dim lands on
    # partitions — the value-identical cast the device otherwise did), and
    # each W as [128, ND, H] fp16 chunked on the contraction dim.
    nc = tc.nc
    x = nc.dram_tensor("x", [BPC, D, S], FP16, kind="ExternalInput").ap()
    wv = nc.dram_tensor("Wv", [P, ND, H], FP16, kind="ExternalInput").ap()
    if USE_FP8_PROJ:
        x8 = nc.dram_tensor("x8", [BPC, D, S], FP8, kind="ExternalInput").ap()
        wq = nc.dram_tensor("Wq", [P, ND, H], FP8, kind="ExternalInput").ap()
        wk = nc.dram_tensor("Wk", [P, ND, H], FP8, kind="ExternalInput").ap()
    else:
        wq = nc.dram_tensor("Wq", [P, ND, H], FP16, kind="ExternalInput").ap()
        wk = nc.dram_tensor("Wk", [P, ND, H], FP16, kind="ExternalInput").ap()
    out = nc.dram_tensor("out", [BPC, S, H], FP32, kind="ExternalOutput").ap()

    import contextlib

    with contextlib.ExitStack() as ctx:
        consts = ctx.enter_context(tc.tile_pool(name="consts", bufs=1))
        big = ctx.enter_context(tc.tile_pool(name="big", bufs=1))
        vpool = ctx.enter_context(tc.tile_pool(name="vpool", bufs=17))
        epool = ctx.enter_context(
            tc.tile_pool(name="epool", bufs=8 if USE_FP8_OUT else 34))
        gpool = ctx.enter_context(tc.tile_pool(name="gpool", bufs=34))
        stage = ctx.enter_context(tc.tile_pool(name="stage", bufs=4))
        small = ctx.enter_context(tc.tile_pool(name="small", bufs=4))
        outp = ctx.enter_context(tc.tile_pool(name="outp", bufs=4))
        psS = ctx.enter_context(tc.tile_pool(name="psS", bufs=4, space="PSUM"))
        psA = ctx.enter_context(tc.tile_pool(name="psA", bufs=4, space="PSUM"))

        ident32 = consts.tile([P, P], FP32)
        make_identity(nc, ident32)
        ident16 = consts.tile([P, P], FP16)
        nc.vector.tensor_copy(ident16, ident32)
        ident8 = consts.tile([P, P], FP8)
        nc.vector.tensor_copy(ident8, ident32)
        identS = ident8 if USE_FP8_SCORES else ident16
        if USE_FP8_OUT:
            ones_row = consts.tile([1, P], FP16, name="ones_row")
            nc.vector.memset(ones_row, 1.0)
            ones_col = consts.tile([P, 1], FP16, name="ones_col")
            nc.vector.memset(ones_col, 1.0)

        # Weights arrive pre-chunked [128, ND, H]; straight DMA.
        QK_DT = FP8 if USE_FP8_PROJ else FP16
        w_sb = []
        for w_ap, nm, dt in ((wq, "wq", QK_DT), (wk, "wk", QK_DT), (wv, "wv", FP16)):
            wt = consts.tile([P, ND, H], dt, name=f"{nm}_sb")
            nc.sync.dma_start(out=wt, in_=w_ap)
            w_sb.append(wt)
        wq_f, wk_f, wv_f = w_sb

        for b in range(BPC):
            # ---- phase 1: xT arrives pre-transposed — straight DMA ----
            # (split by column quarters so the first projection matmuls can
            # start as soon as the first 512 columns land)
            xT = big.tile([P, ND, S], FP16, tag="xT", bufs=2, name=f"xT_{b}")
            xsrc = x[b].rearrange("(c p) s -> p c s", p=P)
            for q4 in range(4):
                nc.sync.dma_start(
                    out=xT[:, :, q4 * GW:(q4 + 1) * GW],
                    in_=xsrc[:, :, q4 * GW:(q4 + 1) * GW])
            if USE_FP8_PROJ:
                xT8 = big.tile([P, ND, S], FP8, tag="xT8", bufs=2, name=f"xT8_{b}")
                x8src = x8[b].rearrange("(c p) s -> p c s", p=P)
                for q4 in range(4):
                    nc.sync.dma_start(
                        out=xT8[:, :, q4 * GW:(q4 + 1) * GW],
                        in_=x8src[:, :, q4 * GW:(q4 + 1) * GW])

            # ---- phase 2: projections, softmax pieces, transposed p/k ----
            pT = big.tile([P, NH, S], SCORES_DT, tag="pT", name=f"pT_{b}")
            kT = big.tile([P, NH, S], SCORES_DT, tag="kT", name=f"kT_{b}")
            rsk_all = small.tile([P, NB], FP16, tag="rsk_all", bufs=2)
            if USE_FP8_OUT:
                c32 = small.tile([P, NB], FP32, tag="c32", bufs=2)
                c16 = small.tile([P, NB], FP16, tag="c16", bufs=2)
                vA_tiles = []
                vB_tiles = []
            v_tiles = []
            eq_tiles = {}
            diag_tiles = {}
            k8_tiles = {}

            def emit_tr(jb):
                tpp = psS.tile([P, H], FP32, tag="s", name="tpp")
                for hc in range(NH):
                    nc.tensor.matmul(
                        tpp[:, hc * P:(hc + 1) * P],
                        eq_tiles[jb][:, hc * P:(hc + 1) * P], diag_tiles[jb],
                        start=True, stop=True)
                nc.any.tensor_copy(
                    out=pT[:, :, jb * P:(jb + 1) * P],
                    in_=tpp.rearrange("p (c f) -> p c f", c=NH))
                tpk = psS.tile([P, H], FP32, tag="s", name="tpk")
                for hc in range(NH):
                    nc.tensor.matmul(
                        tpk[:, hc * P:(hc + 1) * P],
                        k8_tiles[jb][:, hc * P:(hc + 1) * P], identS,
                        start=True, stop=True)
                nc.any.tensor_copy(
                    out=kT[:, :, jb * P:(jb + 1) * P],
                    in_=tpk.rearrange("p (c f) -> p c f", c=NH))

            for ib in range(NB):
                q_ps = psA.tile([P, H], FP32, tag="a", name="q_ps")
                k_ps = psA.tile([P, H], FP32, tag="a", name="k_ps")
                v_ps = psA.tile([P, H], FP32, tag="a", name="v_ps")
                if USE_FP8_PROJ:
                    for ps, wt in ((q_ps, wq_f), (k_ps, wk_f)):
                        for pair in range(ND // 2):
                            nc.tensor.matmul(
                                ps,
                                xT8[:, 2 * pair:2 * pair + 2,
                                    ib * P:(ib + 1) * P],
                                wt[:, 2 * pair:2 * pair + 2, :],
                                start=(pair == 0), stop=(pair == ND // 2 - 1),
                                perf_mode=DR)
                    qk_pairs = ()
                else:
                    qk_pairs = ((q_ps, wq_f), (k_ps, wk_f))
                for ps, wt in qk_pairs + ((v_ps, wv_f),):
                    for dc in range(ND):
                        nc.tensor.matmul(
                            ps, xT[:, dc, ib * P:(ib + 1) * P], wt[:, dc, :],
                            start=(dc == 0), stop=(dc == ND - 1))

                eq_sb = stage.tile([P, H], FP16, tag="eq", bufs=5)
                sq = small.tile([P, 1], FP32, tag="sq")
                nc.scalar.activation(eq_sb, q_ps, EXP, scale=1.0 / QS,
                                     accum_out=sq)
                rq = small.tile([P, 1], FP32, tag="rq")
                nc.vector.reciprocal(rq, sq)
                # diag(PS/sq): folds p-normalization (and the fp8 pre-scale,
                # when enabled) into the p transpose matmul
                diag = stage.tile([P, P], FP16, tag="diag", bufs=5)
                nc.vector.tensor_scalar(
                    diag, ident16, rq, PS,
                    op0=mybir.AluOpType.mult, op1=mybir.AluOpType.mult)

                ek_sb = stage.tile([P, H], FP16, tag="ek", bufs=2)
                sk = small.tile([P, 1], FP32, tag="sk")
                nc.scalar.activation(ek_sb, k_ps, EXP, scale=1.0 / QS,
                                     accum_out=sk)
                rsk = small.tile([P, 1], FP32, tag="rsk")
                nc.vector.reciprocal(rsk, sk)
                v_sb = vpool.tile([P, H], FP16, tag="v")
                nc.vector.tensor_scalar_mul(v_sb, v_ps, rsk)
                if USE_FP8_OUT:
                    # rsk_all holds VS*rsk (the l0 rhs); fp8 v tiles hold
                    # VS*v*rsk in DoubleRow pair layout, with VS*rsk appended
                    # as column 256 of the A half (the denominator column)
                    nc.vector.tensor_scalar_mul(rsk_all[:, ib:ib + 1],
                                                rsk, VS)
                    if ib % 2 == 0:
                        vA = vpool.tile([P, 2, HA], FP8, tag="vA", bufs=9,
                                        name="vA")
                        vB = vpool.tile([P, 2, HB], FP8, tag="vB", bufs=9,
                                        name="vB")
                        vA_tiles.append(vA)
                        vB_tiles.append(vB)
                    m = ib % 2
                    vA, vB = vA_tiles[ib // 2], vB_tiles[ib // 2]
                    nc.vector.tensor_scalar(
                        vA[:, m, 0:256], v_ps[:, 0:256], rsk, VS,
                        op0=mybir.AluOpType.mult, op1=mybir.AluOpType.mult)
                    nc.vector.tensor_scalar(
                        vB[:, m, :], v_ps[:, 256:H], rsk, VS,
                        op0=mybir.AluOpType.mult, op1=mybir.AluOpType.mult)
                    nc.any.tensor_copy(out=vA[:, m, 256:257],
                                       in_=rsk_all[:, ib:ib + 1])
                else:
                    nc.any.tensor_copy(out=rsk_all[:, ib:ib + 1], in_=rsk)
                k8_sb = stage.tile([P, H], SCORES_DT, tag="k8", bufs=5)
                nc.any.tensor_copy(out=k8_sb, in_=k_ps)

                v_tiles.append(v_sb)
                eq_tiles[ib] = eq_sb
                diag_tiles[ib] = diag
                k8_tiles[ib] = k8_sb
                if ib >= 3:
                    emit_tr(ib - 3)

            # ---- phase 3: scores (transposed), exp, output ----
            def emit_scores(gp, jbs, eT, gT):
                igs = (2 * gp, 2 * gp + 1)
                for jb in jbs:
                    s_ps = {ig: psS.tile([P, GW], FP32, tag="s", name="s_ps")
                            for ig in igs}
                    if USE_FP8_SCORES:
                        for pair in range(2):
                            lhs = kT[:, 2 * pair:2 * pair + 2,
                                     jb * P:(jb + 1) * P]
                            for ig in igs:
                                nc.tensor.matmul(
                                    s_ps[ig], lhs,
                                    pT[:, 2 * pair:2 * pair + 2,
                                       ig * GW:(ig + 1) * GW],
                                    start=(pair == 0), stop=(pair == 1),
                                    perf_mode=DR)
                    else:
                        for hc in range(NH):
                            lhs = kT[:, hc, jb * P:(jb + 1) * P]
                            for ig in igs:
                                nc.tensor.matmul(
                                    s_ps[ig], lhs,
                                    pT[:, hc, ig * GW:(ig + 1) * GW],
                                    start=(hc == 0), stop=(hc == NH - 1))
                    for ig in igs:
                        e_sb = epool.tile([P, GW], FP16, tag="e")
                        if USE_FP8_OUT:
                            acc = None
                            if gp == 0 and ig == 0:
                                acc = small.tile([P, 1], FP32, tag="acc")
                            nc.scalar.activation(
                                e_sb, s_ps[ig], EXP, scale=1.0 / (PS * QS),
                                accum_out=acc)
                            if acc is not None:
                                nc.vector.tensor_scalar_mul(
                                    c32[:, jb:jb + 1], acc, 1.0 / GW)
                                nc.vector.tensor_copy(
                                    c16[:, jb:jb + 1], c32[:, jb:jb + 1])
                            if jb % 2 == 0:
                                gT[ig][jb // 2] = gpool.tile(
                                    [P, 2, GW], FP8, tag="g",
                                    name=f"g_{ig}_{jb // 2}")
                            nc.vector.tensor_scalar(
                                gT[ig][jb // 2][:, jb % 2, :], e_sb,
                                c32[:, jb:jb + 1], None,
                                op0=mybir.AluOpType.subtract)
                        else:
                            nc.scalar.activation(e_sb, s_ps[ig], EXP,
                                                 scale=1.0 / (PS * QS))
                        eT[ig][jb] = e_sb

            def emit_w():
                # Bulk paths of the c+g split: w = sum_j c_j vr16_j and
                # l0 = sum_j c_j (VS*rsk16)_j, interleaved so the N=1 l0
                # matmuls' weight loads hide under the w streams (same lhs).
                w_ps = psA.tile([1, H], FP32, tag="a", name="w_ps")
                l0_ps = psA.tile([1, 1], FP32, tag="a", name="l0_ps")
                for jc in range(NB):
                    nc.tensor.matmul(w_ps, c16[:, jc:jc + 1], v_tiles[jc],
                                     start=(jc == 0), stop=(jc == NB - 1))
                    nc.tensor.matmul(l0_ps, c16[:, jc:jc + 1],
                                     rsk_all[:, jc:jc + 1],
                                     start=(jc == 0), stop=(jc == NB - 1))
                wA_sb = small.tile([1, HA], FP16, tag="wA_sb", bufs=2)
                wB_sb = small.tile([1, HB], FP16, tag="wB_sb", bufs=2)
                nc.vector.tensor_scalar_mul(wA_sb[:, 0:256], w_ps[:, 0:256],
                                            VS)
                nc.vector.tensor_copy(wA_sb[:, 256:257], l0_ps)
                nc.vector.tensor_scalar_mul(wB_sb, w_ps[:, 256:H], VS)
                return wA_sb, wB_sb

            def emit_out_block(ig, il, gT, wAB):
                ib = ig * NG + il
                wA_sb, wB_sb = wAB
                oA = psA.tile([P, HA], FP32, tag="a", name="oA")
                oB = psA.tile([P, HB], FP32, tag="a", name="oB")
                nc.tensor.matmul(oA, ones_row, wA_sb, start=True, stop=False)
                nc.tensor.matmul(oB, ones_row, wB_sb, start=True, stop=False)
                for jp in range(NB // 2):
                    lhs = gT[ig][jp][:, :, il * P:(il + 1) * P]
                    nc.tensor.matmul(oA, lhs, vA_tiles[jp], start=False,
                                     stop=(jp == NB // 2 - 1), perf_mode=DR)
                    nc.tensor.matmul(oB, lhs, vB_tiles[jp], start=False,
                                     stop=(jp == NB // 2 - 1), perf_mode=DR)
                rl = small.tile([P, 1], FP32, tag="rl")
                nc.vector.reciprocal(rl, oA[:, 256:257])
                o_sb = outp.tile([P, H], FP32, tag="o")
                nc.vector.tensor_scalar_mul(o_sb[:, 0:256], oA[:, 0:256], rl)
                nc.vector.tensor_scalar_mul(o_sb[:, 256:H], oB, rl)
                nc.sync.dma_start(
                    out=out[b, ib * P:(ib + 1) * P, :], in_=o_sb)

            def emit_out(gp, eT, gT, wAB):
                igs = (2 * gp, 2 * gp + 1)
                for ig in igs:
                    for il in range(NG):
                        if USE_FP8_OUT:
                            emit_out_block(ig, il, gT, wAB)
                            continue
                        ib = ig * NG + il
                        o_ps = psA.tile([P, H], FP32, tag="a", name="o_ps")
                        l_ps = psA.tile([P, 1], FP32, tag="a", name="l_ps")
                        for jc in range(NB):
                            lhs = eT[ig][jc][:, il * P:(il + 1) * P]
                            nc.tensor.matmul(
                                o_ps, lhs, v_tiles[jc],
                                start=(jc == 0), stop=(jc == NB - 1))
                            nc.tensor.matmul(
                                l_ps, lhs, rsk_all[:, jc:jc + 1],
                                start=(jc == 0), stop=(jc == NB - 1))
                        rl = small.tile([P, 1], FP32, tag="rl")
                        nc.vector.reciprocal(rl, l_ps)
                        o_sb = outp.tile([P, H], FP32, tag="o")
                        nc.vector.tensor_scalar_mul(o_sb, o_ps, rl)
                        nc.sync.dma_start(
                            out=out[b, ib * P:(ib + 1) * P, :], in_=o_sb)

            # Hoist the first 13 gp0 score blocks (inputs ready: their kT/pT
            # transposes are long done) in front of the tail transposes so the
            # latter never stall the PE on the Scalar engine's exp backlog.
            eT0 = {0: {}, 1: {}}
            gT0 = {0: {}, 1: {}}
            emit_scores(0, range(NB - 3), eT0, gT0)
            emit_tr(NB - 3)
            emit_tr(NB - 2)
            emit_tr(NB - 1)
            emit_scores(0, range(NB - 3, NB), eT0, gT0)
            eT1 = {2: {}, 3: {}}
            gT1 = {2: {}, 3: {}}
            if USE_FP8_OUT:
                # Feed the in-order PE queue gp1 score work (long-ready
                # inputs) while gp0's exp->subtract chains drain, then
                # interleave gp0 output blocks with the remaining gp1 jbs.
                emit_scores(1, range(4), eT1, gT1)
                wAB = emit_w()
                blocks0 = [(ig, il) for ig in (0, 1) for il in range(NG)]
                nxt = 4
                for bi, (ig, il) in enumerate(blocks0):
                    emit_out_block(ig, il, gT0, wAB)
                    take = min(2, NB - nxt)
                    if take:
                        emit_scores(1, range(nxt, nxt + take), eT1, gT1)
                        nxt += take
                emit_out(1, eT1, gT1, wAB)
            else:
                emit_out(0, eT0, gT0, None)
                emit_scores(1, range(NB), eT1, gT1)
                emit_out(1, eT1, gT1, None)


_NC_CACHE = {}


def _get_nc():
    if "nc" not in _NC_CACHE:
        nc = bacc.Bacc("TRN2", target_bir_lowering=False, debug=False)
        with tile.TileContext(nc) as tc:
            _emit(tc)
        nc.compile()
        _NC_CACHE["nc"] = nc
    return _NC_CACHE["nc"]


def _prep_w(w, dtype=np.float16, scale=1.0):
    # [D, H] fp32 -> [128, ND, H] chunked on the contraction dim
    w = np.asarray(w, dtype=np.float32).reshape(ND, P, H).transpose(1, 0, 2)
    if scale != 1.0:
        w = w * scale
    return np.ascontiguousarray(w).astype(dtype)


def _prep_x_shard(xs, dtype=np.float16):
    # [BPC, S, D] fp32 -> [BPC, D, S] (contraction dim on partitions)
    return np.ascontiguousarray(xs.transpose(0, 2, 1)).astype(dtype)


def _run(inputs, trace=False, trace_cores=None):
    import ml_dtypes

    nc = _get_nc()
    x = np.asarray(inputs["x"], dtype=np.float32)
    f8 = ml_dtypes.float8_e4m3
    qk_dt = f8 if USE_FP8_PROJ else np.float16
    qk_scale = WS if USE_FP8_PROJ else 1.0
    wq = _prep_w(inputs["Wq"], qk_dt, qk_scale)
    wk = _prep_w(inputs["Wk"], qk_dt, qk_scale)
    wv = _prep_w(inputs["Wv"])
    in_maps = []
    for c in range(NCORES):
        xs = x[c * BPC:(c + 1) * BPC]
        m = {"x": _prep_x_shard(xs), "Wq": wq, "Wk": wk, "Wv": wv}
        if USE_FP8_PROJ:
            m["x8"] = _prep_x_shard(xs, f8)
        in_maps.append(m)
    res = run_bass_kernel_spmd(
        nc, in_maps, core_ids=list(range(NCORES)),
        trace=trace, trace_cores=trace_cores)
    full = np.concatenate([res.results[c]["out"] for c in range(NCORES)], axis=0)
    return full, res


def kernel(**inputs) -> np.ndarray:
    out, _ = _run(inputs)
    return out

